# revision 27
# baseline (speedup 1.0000x reference)
"""MLA (multi-head latent) causal attention on 8 Trainium2 NeuronCores.

Sharding: batch(4) x head-group(2) mesh over 8 cores. Core c handles batch
c//2 and heads [8*(c%2), 8*(c%2)+8). The latent KV projections are small and
recomputed per head-group (an MLA property: the latent KV is shared across
heads). Each core produces a partial output (its head-group's contribution to
y @ wo^T for its batch); the host sums the two partials per batch.

All data is bf16 (rel tolerance 2e-2 leaves plenty of room): this enables the
PE's Fast Weight Load path, halves HBM traffic, and doubles DVE throughput.
PSUM accumulation stays fp32.

Single fused loop over the 4 query chunks of 512 tokens; everything streams
through SBUF (no DRAM scratch):
  per chunk n:
    h-loop (software pipelined): q-proj(h) chain -> RoPE(h) on ACT+DVE,
      scores(h-1) blocks k_j^T q -> causal tri add -> exp on ACT -> bf16
      partial sums of exp on DVE (softmax denominator), PV + one small
      ones-matmul (z) + 1/z normalize of head h-2 interleaved mid-scores.
    tail: latent kv proj for chunk n+1 split around the zpv flush (covers
      the exp/normalize latency of the last head), then the output
      projection out[t,c] += y_h^T wo_h for chunk n's 4 token-128 blocks.
"""

import math
from contextlib import ExitStack

import numpy as np

import concourse.bass as bass
import concourse.mybir as mybir
import concourse.tile as tile
from concourse import bacc
from concourse.bass_utils import run_bass_kernel_spmd

# Problem shape (hardcoded per contract).
B, T, C = 4, 2048, 2048
H, D, L = 16, 128, 512
HG = 8           # heads per core
N_CORES = 8
P = 128
KC = C // P      # 16 contraction chunks over C
LC = L // P      # 4 chunks over L
NQ = T // 512    # 4 query chunks of 512
NT = T // P      # 16 key chunks of 128
SCALE = 1.0 / math.sqrt(D)

F32 = mybir.dt.float32
BF16 = mybir.dt.bfloat16
NPBF16 = mybir.dt.np(BF16)

_cached = {}


def _build_program():
    nc = bacc.Bacc()

    xT = nc.dram_tensor("xT", [C, T], BF16, kind="ExternalInput").ap()
    wqT = nc.dram_tensor("wqT", [C, HG * D], BF16, kind="ExternalInput").ap()
    wkvT = nc.dram_tensor("wkvT", [C, L], BF16, kind="ExternalInput").ap()
    wkuT = nc.dram_tensor("wkuT", [L, D], BF16, kind="ExternalInput").ap()
    wvuT = nc.dram_tensor("wvuT", [L, D], BF16, kind="ExternalInput").ap()
    woT = nc.dram_tensor("woT", [HG * D, C], BF16, kind="ExternalInput").ap()
    c2 = nc.dram_tensor("c2", [P, T], BF16, kind="ExternalInput").ap()
    s2 = nc.dram_tensor("s2", [P, T], BF16, kind="ExternalInput").ap()
    outp = nc.dram_tensor("outp", [T, C], BF16, kind="ExternalOutput").ap()

    xT_r = xT.rearrange("(kc p) t -> p kc t", p=P)
    wqT_r = wqT.rearrange("(kc p) e -> p kc e", p=P)
    wkvT_r = wkvT.rearrange("(kc p) l -> p kc l", p=P)
    wkuT_r = wkuT.rearrange("(lc p) d -> p lc d", p=P)
    wvuT_r = wvuT.rearrange("(lc p) d -> p lc d", p=P)
    woT_r = woT.rearrange("(h p) c -> p h c", p=P)

    with tile.TileContext(nc) as tc, ExitStack() as top:
        persist = top.enter_context(tc.tile_pool(name="persist", bufs=1))
        pall = top.enter_context(tc.tile_pool(name="pall", bufs=8, space="PSUM"))
        xpool = top.enter_context(tc.tile_pool(name="xpool", bufs=2))
        kvpool = top.enter_context(tc.tile_pool(name="kvpool", bufs=1))
        rpool = top.enter_context(tc.tile_pool(name="rpool", bufs=2))
        qtp = top.enter_context(tc.tile_pool(name="qtp", bufs=3))
        epool = top.enter_context(tc.tile_pool(name="epool", bufs=2))
        zpool = top.enter_context(tc.tile_pool(name="zpool", bufs=2))
        ypool = top.enter_context(tc.tile_pool(name="ypool", bufs=2))
        opool = top.enter_context(tc.tile_pool(name="opool", bufs=1))

        k_slab = persist.tile([P, NT, P], BF16)      # k_rot^T: [d, ts_chunk, ts]
        v_slab = persist.tile([P, NT, P], BF16)      # v: [ts, ts_chunk, d]
        tri = persist.tile([P, P], F32)              # additive causal triangle
        ones = persist.tile([P, P], BF16)
        ones_f = persist.tile([P, 1], F32)
        c2_sb = persist.tile([P, T], BF16)
        s2_sb = persist.tile([P, T], BF16)
        wku_sb = persist.tile([P, LC, D], BF16)
        wvu_sb = persist.tile([P, LC, D], BF16)
        wq_sb = persist.tile([P, KC, HG * P], BF16)
        wo_sb = persist.tile([P, HG, C], BF16)

        nc.vector.memset(ones_f[:], 1.0)
        nc.vector.tensor_copy(ones[:], ones_f[:].to_broadcast([P, P]))
        nc.gpsimd.memset(tri[:], 0.0)
        nc.gpsimd.affine_select(
            out=tri[:], in_=tri[:],
            compare_op=mybir.AluOpType.is_ge,
            fill=-1e9, base=0,
            pattern=[[1, P]], channel_multiplier=-1,
        )

        # prologue DMAs; x(0) + wkv first (they feed kv(0), the first PE work)
        # -- the big wq/wo loads are queued after so they don't starve it
        xts = [None] * NQ
        xts[0] = xpool.tile([P, KC, 512], BF16, tag="x", name="x0")
        wkv_sb = persist.tile([P, KC, L], BF16)
        # interleave x(0) / wkv in kc-group order so the first kv matmuls
        # (kc-ordered) can start after ~1/4 of the bytes land
        # issue the startup loads from three idle engine queues in parallel
        # so the first kv matmuls aren't gated on serial DMA dispatch
        nc.sync.dma_start(xts[0][:, 0, :], xT_r[:, 0, bass.ts(0, 512)])
        nc.scalar.dma_start(wkv_sb[:, 0, :], wkvT_r[:, 0, :])
        nc.sync.dma_start(xts[0][:, 1:4, :],
                          xT_r[:, 1:4, bass.ts(0, 512)])
        nc.scalar.dma_start(wkv_sb[:, 1:4, :], wkvT_r[:, 1:4, :])
        for g in range(1, 4):
            nc.sync.dma_start(xts[0][:, bass.ts(g, 4), :],
                              xT_r[:, bass.ts(g, 4), bass.ts(0, 512)])
            nc.scalar.dma_start(wkv_sb[:, bass.ts(g, 4), :],
                                wkvT_r[:, bass.ts(g, 4), :])
        nc.sync.dma_start(c2_sb[:], c2)
        nc.sync.dma_start(s2_sb[:], s2)
        nc.sync.dma_start(wku_sb[:], wkuT_r)
        nc.sync.dma_start(wvu_sb[:], wvuT_r)

        state = {"pending": None}
        qts = [None] * HG

        def rope_copy(ps):
            # single PSUM read via ACT so the bank frees immediately
            qq = rpool.tile([P, 512], BF16, tag="qq")
            nc.scalar.copy(qq[:], ps[:])
            return qq

        def rope_mults(dst, qq, n):
            # dst = qq * c2 + swap64(qq) * s2 on DVE (the two-input TT ops
            # require equal base partitions, so the swap stays a copy)
            c2n = c2_sb[:, bass.ts(n, 512)]
            s2n = s2_sb[:, bass.ts(n, 512)]
            qs = rpool.tile([P, 512], BF16, tag="qs")
            nc.vector.tensor_copy(qs[0:64, :], qq[64:128, :])
            nc.vector.tensor_copy(qs[64:128, :], qq[0:64, :])
            nc.vector.tensor_tensor(qs[:], qs[:], s2n, mybir.AluOpType.mult)
            nc.vector.tensor_tensor(qq[:], qq[:], c2n, mybir.AluOpType.mult)
            nc.vector.tensor_tensor(dst, qq[:], qs[:], mybir.AluOpType.add)

        def rope(dst, ps, n):
            rope_mults(dst, rope_copy(ps), n)

        def flush_zpv():
            if state["pending"] is None:
                return
            n, h, nts, spans, exp_t, zacc, y_t = state["pending"]
            state["pending"] = None
            yp = pall.tile([P, 512], F32, tag="pa", name=f"yp{n}_{h}")
            for j in range(nts):
                sl = slice(spans[j], 512)
                nc.tensor.matmul(yp[:, sl], v_slab[:, j, :], exp_t[:, j, sl],
                                 start=(j == 0), stop=(j == nts - 1))
            # z = column sums of the DVE-accumulated exp partials: one small
            # ones-matmul instead of one per key block
            zp = pall.tile([P, 512], F32, tag="pa", name=f"zp{n}_{h}")
            nc.tensor.matmul(zp[:], ones[:], zacc[:], start=True, stop=True)
            zr = zpool.tile([P, 512], F32, tag="zr")
            nc.vector.reciprocal_approx_fast(out=zr[:], in_=zp[:])
            nc.vector.tensor_tensor(y_t[:, h, :], yp[:], zr[:],
                                    mybir.AluOpType.mult)

        def emit_qproj_mms(n, h):
            # q projection chain + the PSUM-freeing ACT copy; the DVE rope
            # multiplies are emitted separately at the end of the iteration so
            # they queue BEHIND the previous head's mask/z-adds on DVE
            # (emitting them first head-of-line-blocks the DVE queue)
            qp = pall.tile([P, 512], F32, tag="pa", name=f"qp{n}_{h}")
            for kc in range(KC):
                nc.tensor.matmul(qp[:], wq_sb[:, kc, bass.ts(h, P)],
                                 xts[n][:, kc, :],
                                 start=(kc == 0), stop=(kc == KC - 1))
            return rope_copy(qp)

        def emit_rope_q(n, h, qq):
            qt = qtp.tile([P, 512], BF16, tag="qt", name=f"q{n}_{h}")
            rope_mults(qt[:], qq, n)
            qts[h] = qt

        def emit_scores(n, h, y_t, mid_cb=None):
            nts = 4 * n + 4
            spans = [max(P * j - 512 * n, 0) for j in range(nts)]
            exp_t = epool.tile([P, NT, 512], BF16, tag="exp", name=f"e{n}_{h}")
            q_t = qts[h]
            # bf16 partial sums of exp over key blocks on DVE: unmasked blocks
            # pairwise as [P,2,512] flat ops, the 4 diagonal blocks into a
            # separate accumulator, folded at the end
            zg2 = zpool.tile([P, 2, 512], BF16, tag="zg2", name=f"zg2_{n}_{h}")
            zgm = zpool.tile([P, 512], BF16, tag="zgm", name=f"zgm{n}_{h}")
            flush_at = 4 if nts > 4 else 0
            for j in range(nts):
                if j == flush_at:
                    flush_zpv()
                    if mid_cb is not None:
                        mid_cb()
                        mid_cb = None
                g = spans[j]
                sl = slice(g, 512)
                scp = pall.tile([P, 512], F32, tag="pa", name=f"sc{n}_{h}_{j}")
                nc.tensor.matmul(scp[:, sl], k_slab[:, j, :], q_t[:, sl],
                                 start=True, stop=True)
                if j >= 4 * n:
                    nc.vector.tensor_tensor(
                        scp[:, g:g + P], scp[:, g:g + P], tri[:],
                        mybir.AluOpType.add)
                nc.scalar.activation(
                    exp_t[:, j, sl], scp[:, sl],
                    mybir.ActivationFunctionType.Exp, scale=SCALE)
                if j < 4 * n:
                    if j % 2 == 1:
                        pair = exp_t[:, j - 1:j + 1, :]
                        if j == 1:
                            nc.vector.tensor_copy(zg2[:], pair)
                        else:
                            nc.vector.tensor_tensor(zg2[:], zg2[:], pair,
                                                    mybir.AluOpType.add)
                else:
                    if j == 4 * n:
                        nc.vector.tensor_copy(zgm[:], exp_t[:, j, :])
                    else:
                        nc.vector.tensor_tensor(zgm[:, sl], zgm[:, sl],
                                                exp_t[:, j, sl],
                                                mybir.AluOpType.add)
            if n > 0:
                nc.vector.tensor_tensor(zgm[:], zgm[:], zg2[:, 0, :],
                                        mybir.AluOpType.add)
                nc.vector.tensor_tensor(zgm[:], zgm[:], zg2[:, 1, :],
                                        mybir.AluOpType.add)
            if flush_at == 0 and state["pending"] is not None:
                flush_zpv()
            if mid_cb is not None:
                mid_cb()
            state["pending"] = (n, h, nts, spans, exp_t, zgm, y_t)

        kvps_st = {}

        def emit_kv_head(n, kcs):
            # latent kv for chunk n, contraction sub-range kcs
            if n not in kvps_st:
                kvps_st[n] = [pall.tile([P, 512], F32, tag="pa",
                                        name=f"kv{n}_{i}") for i in range(LC)]
            for kc in kcs:
                for lc in range(LC):
                    nc.tensor.matmul(kvps_st[n][lc][:],
                                     wkv_sb[:, kc, bass.ts(lc, P)],
                                     xts[n][:, kc, :],
                                     start=(kc == 0), stop=(kc == KC - 1))

        def emit_kv_tail(n):
            kvps = kvps_st.pop(n)
            kvn = kvpool.tile([P, LC, 512], BF16, tag="kvn")
            for lc in range(LC):
                nc.scalar.copy(kvn[:, lc, :], kvps[lc][:])

            kp = pall.tile([P, 512], F32, tag="pa", name=f"kp{n}")
            for lc in range(LC):
                nc.tensor.matmul(kp[:], wku_sb[:, lc, :], kvn[:, lc, :],
                                 start=(lc == 0), stop=(lc == LC - 1))
            kdst = k_slab[:, 4 * n:4 * (n + 1), :].rearrange("p a b -> p (a b)")
            rope(kdst, kp, n)

            for i in range(4):
                vp = pall.tile([P, P], F32, tag="pa", name=f"vp{n}_{i}")
                for lc in range(LC):
                    nc.tensor.matmul(vp[:], kvn[:, lc, bass.ts(i, P)],
                                     wvu_sb[:, lc, :],
                                     start=(lc == 0), stop=(lc == LC - 1))
                nc.scalar.copy(v_slab[:, 4 * n + i, :], vp[:])

        def emit_C(n, y_t):
            # h-inner with ci under it: the 4 ci matmuls share the y-chunk
            # stationary; drains on ACT (idle here); one contiguous DMA per
            # 128-token row block
            for t in range(4):
                ops = [pall.tile([P, 512], F32, tag="pa",
                                 name=f"op{n}_{t}_{ci}") for ci in range(4)]
                for h in range(HG):
                    for ci in range(4):
                        nc.tensor.matmul(ops[ci][:], y_t[:, h, bass.ts(t, P)],
                                         wo_sb[:, h, bass.ts(ci, 512)],
                                         start=(h == 0), stop=(h == HG - 1))
                ost = opool.tile([P, 4, 512], BF16, tag="ost")
                for ci in range(4):
                    nc.vector.tensor_copy(ost[:, ci, :], ops[ci][:])
                nc.sync.dma_start(
                    outp[bass.ts(4 * n + t, P), :],
                    ost[:].rearrange("p a b -> p (a b)"))

        emit_kv_head(0, range(KC))
        emit_kv_tail(0)
        for i in range(4):
            nc.sync.dma_start(wq_sb[:, :, bass.ts(i, HG * P // 4)],
                              wqT_r[:, :, bass.ts(i, HG * P // 4)])
        for n in range(NQ):
            if n == 0:
                for i in range(4):
                    nc.sync.dma_start(wo_sb[:, :, bass.ts(i, C // 4)],
                                      woT_r[:, :, bass.ts(i, C // 4)])
            if n + 1 < NQ:
                xts[n + 1] = xpool.tile([P, KC, 512], BF16, tag="x",
                                        name=f"x{n + 1}")
                nc.sync.dma_start(xts[n + 1][:],
                                  xT_r[:, :, bass.ts(n + 1, 512)])
            y_t = ypool.tile([P, HG, 512], BF16, tag="yc", name=f"y{n}")
            for h in range(HG):
                qq = emit_qproj_mms(n, h)
                if h >= 1:
                    emit_scores(n, h - 1, y_t,
                                mid_cb=(lambda h=h, qq=qq:
                                        emit_rope_q(n, h, qq)))
                else:
                    emit_rope_q(n, h, qq)
            # tail: first half of kv(n+1) covers the rope(h7) latency before
            # scores(h7); the second half covers exp(h7)+normalize so the zpv
            # flush and emit_C start stall-free
            if n + 1 < NQ:
                emit_kv_head(n + 1, range(0, KC // 2))
                emit_scores(n, HG - 1, y_t)
                emit_kv_head(n + 1, range(KC // 2, KC))
                flush_zpv()
                emit_kv_tail(n + 1)
            else:
                emit_scores(n, HG - 1, y_t)
                flush_zpv()
            emit_C(n, y_t)

    nc.finalize()
    return nc


_PERM = np.concatenate([np.arange(0, D, 2), np.arange(1, D, 2)])


def _prep_core_inputs(x, freqs_cos, freqs_sin, wq, wkv_down, wk_up, wv_up, wo):
    cosT = np.ascontiguousarray(freqs_cos.T)                      # [64, T]
    sinT = np.ascontiguousarray(freqs_sin.T)
    c2 = np.concatenate([cosT, cosT], axis=0).astype(NPBF16)      # [128, T]
    s2 = np.concatenate([-sinT, sinT], axis=0).astype(NPBF16)

    wkvT = np.ascontiguousarray(wkv_down.T).astype(NPBF16)        # [C, L]
    wkuT = np.ascontiguousarray(wk_up[_PERM, :].T).astype(NPBF16)  # [L, D]
    wvuT = np.ascontiguousarray(wv_up.T).astype(NPBF16)           # [L, D]

    wq_h = wq.reshape(H, D, C)[:, _PERM, :]                       # perm rows/head

    in_maps = []
    for core in range(N_CORES):
        b, g = core // 2, core % 2
        heads = slice(8 * g, 8 * g + 8)
        wqT_g = np.ascontiguousarray(
            wq_h[heads].reshape(HG * D, C).T).astype(NPBF16)      # [C, 1024]
        woT_g = np.ascontiguousarray(
            wo[:, 8 * g * D:(8 * g + 8) * D].T).astype(NPBF16)    # [1024, C]
        xT_b = np.ascontiguousarray(x[b].T).astype(NPBF16)        # [C, T]
        in_maps.append({
            "xT": xT_b, "wqT": wqT_g, "wkvT": wkvT, "wkuT": wkuT,
            "wvuT": wvuT, "woT": woT_g, "c2": c2, "s2": s2,
        })
    return in_maps


def kernel(x, freqs_cos, freqs_sin, wq, wkv_down, wk_up, wv_up, wo, _trace=False):
    x = np.asarray(x, dtype=np.float32)
    freqs_cos = np.asarray(freqs_cos, dtype=np.float32)
    freqs_sin = np.asarray(freqs_sin, dtype=np.float32)
    wq = np.asarray(wq, dtype=np.float32)
    wkv_down = np.asarray(wkv_down, dtype=np.float32)
    wk_up = np.asarray(wk_up, dtype=np.float32)
    wv_up = np.asarray(wv_up, dtype=np.float32)
    wo = np.asarray(wo, dtype=np.float32)

    if "nc" not in _cached:
        _cached["nc"] = _build_program()
    nc = _cached["nc"]

    in_maps = _prep_core_inputs(x, freqs_cos, freqs_sin, wq, wkv_down,
                                wk_up, wv_up, wo)
    res = run_bass_kernel_spmd(nc, in_maps, core_ids=list(range(N_CORES)),
                               trace=_trace)
    _cached["last_result"] = res

    out = np.empty((B, T, C), dtype=np.float32)
    for b in range(B):
        out[b] = (res.results[2 * b]["outp"].astype(np.float32)
                  + res.results[2 * b + 1]["outp"].astype(np.float32))
    return out
